# revision 37
# baseline (speedup 1.0000x reference)
"""Trainium2 Bass kernel for nn_MultiHeadAttention (B=4, C=1024, T=1024, H=16).

Sharding: 8 cores = (batch b in 0..3) x (head-group g in 0..1), 8 heads per
group. Each core computes q/k/v projections for its group's 512 channels,
rope, attention, and a partial O-projection Wo[:, group] @ att. The host sums
the two partials per batch (bias bo is supplied only to g=0 cores).

v2 design (cost model: matmul cost = out-free-size x 1 cycle per 128-K-chunk;
fp16 keeps 1 cycle/row at any output width):
  - everything fp16 on the wire and in SBUF (fp32 PSUM), halving DMA bytes;
    fp16 over bf16 for the 3 extra mantissa bits at identical cost.
  - scores computed transposed per head: scoresT[s, t] = k[d,s].T @ q[d,t],
    two heads (64 channels each) per 128-partition m-tile.
  - exp on ScalarE straight from PSUM in [128, 1024] tiles (both heads of a
    pair at once) with the 1/sqrt(hd) scale fused; output p in fp16.
  - PV runs TRANSPOSED: out[t, d] = p[s,t].T @ v[s,d] per (head, 128-col
    t-chunk), using all 128 PSUM partitions (65-wide outputs, fp16 1 c/r) --
    half the PE cost of the [65, t] orientation. The softmax denominator is
    an extra ones-column in v (column 64).
  - normalization: reciprocal of pv[:, 64] ([128,1]) + tensor_scalar_mul
    per head/t-chunk -- no gpsimd broadcast needed.
  - attT -> att ([c, t]) via dma_start_transpose (DMA xbar, 14ns per 16x128
    tile -- off the PE entirely), feeding a standard O-projection.
  - q/k bias adds ride on ScalarE (activation Identity + bias) during the
    projection phase when it is otherwise idle.
  - software pipeline: the exp stream is the attention-phase bottleneck, so
    independent PE work (k-projection m-tiles, q j1-half m-tiles, rope perm
    matmuls, O-projection units) is interleaved INTO the per-s-tile loop as
    "fillers" between the score matmuls and the exp-dependent PV matmuls.
"""
import sys
import time

sys.path.insert(0, '/opt/trn_rl_repo')

import numpy as np
import ml_dtypes

B = 4
C = 1024
T = 1024
H = 16
HD = C // H            # 64
D_ROPE = HD // 2       # 32
HALF = D_ROPE // 2     # 16
GROUPS = 2
NCORES = 8
NH = H // GROUPS       # 8 heads per group
CHG = NH * HD          # 512 channels per group
KT = C // 128          # 8 input-channel k-tiles
ST = T // 128          # 8 s-tiles
TC = 512
NT = T // TC           # 2 t-chunks
NT2 = TC // 128        # 4 128-col sub-chunks per t-chunk
MT = CHG // 128        # 4 projection m-tiles per group
OMT = C // 128         # 8 output m-tiles
NPAIR = NH // 2        # 4 head-pairs (2 heads packed per 128-tile)
SCALE = 0.125          # 1/sqrt(HD)
BF = np.float16

_cache = {}


def _rope_tables():
    theta = 1.0 / (10000.0 ** (np.arange(HALF, dtype=np.float64) * 2.0 / D_ROPE))
    ang = np.arange(T, dtype=np.float64)[:, None] * theta[None, :]   # [T, HALF]
    cos = np.concatenate([np.cos(ang), np.cos(ang)], axis=1)         # [T, D_ROPE]
    sin = np.concatenate([np.sin(ang), np.sin(ang)], axis=1)
    return cos.astype(np.float32), sin.astype(np.float32)


def _cs_tiles():
    """C,S tables in [128 ch, T] layout; the 2-head (64-row) pattern repeats,
    so one 128-row tile serves every projection m-tile."""
    cos, sin = _rope_tables()
    Ct = np.ones((128, T), dtype=np.float32)
    St = np.zeros((128, T), dtype=np.float32)
    for h in range(2):
        o = h * HD
        Ct[o:o + D_ROPE, :] = cos.T
        St[o:o + D_ROPE, :] = sin.T
    return Ct, St


def _perm_matrix():
    """Signed rope permutation acting on a 128-row (2-head) tile:
    y[d] = -x[d+16] (d<16), x[d-16] (16<=d<32), 0 otherwise; lhsT layout."""
    P = np.zeros((128, 128), dtype=np.float32)
    for o in (0, 64):
        for d in range(HALF):
            P[o + d, o + d + HALF] = -1.0
            P[o + d + HALF, o + d] = 1.0
    return np.ascontiguousarray(P.T)


def _build_nc():
    import concourse.tile as tile
    from concourse import bacc, mybir

    F32 = mybir.dt.float32
    F16 = mybir.dt.float16
    AF = mybir.ActivationFunctionType

    nc = bacc.Bacc(name="mha")
    dram = {}
    for name, shape, dt in [
        ("x", (C, T), F16), ("cc", (C, T), F16),
        ("wqT", (C, CHG), F16), ("wkT", (C, CHG), F16),
        ("wvT", (C, CHG), F16), ("woT", (CHG, C), F16),
        ("bq", (CHG, 1), F32), ("bk", (CHG, 1), F32),
        ("bv", (1, CHG), F32), ("bo", (C, 1), F32),
        ("Ct", (128, T), F16), ("St", (128, T), F16),
        ("permT", (128, 128), F16), ("ones128", (128, 1), F32),
    ]:
        dram[name] = nc.dram_tensor(name, shape, dt, kind="ExternalInput")
    out = nc.dram_tensor("out", (C, T), F32, kind="ExternalOutput")

    with tile.TileContext(nc) as tc:
        with tc.tile_pool(name="io", bufs=1) as io, \
             tc.tile_pool(name="wp", bufs=3) as wpool, \
             tc.tile_pool(name="sc", bufs=2) as spool, \
             tc.tile_pool(name="pp", bufs=4) as ppool, \
             tc.tile_pool(name="ob", bufs=8) as opool, \
             tc.tile_pool(name="psq", bufs=2, space="PSUM") as psq, \
             tc.tile_pool(name="pss", bufs=2, space="PSUM") as pss, \
             tc.tile_pool(name="pv", bufs=1, space="PSUM") as pvp:

            # ---------- resident tiles ----------
            xt = io.tile([128, KT, T], F16, tag="x")
            ct = io.tile([128, KT, T], F16, tag="c")
            qr = io.tile([128, MT, T], F16, tag="qr")
            kr = io.tile([128, MT, T], F16, tag="kr")
            att = io.tile([128, MT, T], F16, tag="att")
            Ctt = io.tile([128, T], F16, tag="Ct")
            Stt = io.tile([128, T], F16, tag="St")
            permTt = io.tile([128, 128], F16, tag="permT")
            ones_col = io.tile([128, 1], F32, tag="ones_col")
            bv_bc = io.tile([128, CHG], F32, tag="bv_bc")

            def half_load(dst, src, ksl, tsl=None):
                """Row-block [512, w] DRAM -> [128, 4, w] SBUF k-tile DMA."""
                s = src[ksl.start * 128:ksl.stop * 128]
                if tsl is not None:
                    s = s[:, tsl]
                    d = dst[:, ksl, tsl]
                else:
                    d = dst[:, ksl]
                nc.sync.dma_start(d, s.rearrange("(k p) c -> p k c", p=128))

            # ---------- DMA stream (ordered for earliest PE start) ----------
            wq = wpool.tile([128, KT, CHG], F16, tag="wres", name="wq")
            half_load(wq, dram["wqT"], slice(0, 4))
            half_load(xt, dram["x"], slice(0, 4), slice(0, TC))
            half_load(wq, dram["wqT"], slice(4, 8))
            half_load(xt, dram["x"], slice(4, 8), slice(0, TC))
            bcol = {}
            for bn, nmt in (("bq", MT), ("bk", MT)):
                bcol[bn] = io.tile([128, nmt, 1], F32, tag=bn, name=bn)
                nc.sync.dma_start(bcol[bn][:],
                                  dram[bn].rearrange("(mt p) o -> p mt o", p=128))
            nc.sync.dma_start(permTt[:], dram["permT"][:])
            nc.sync.dma_start(Ctt[:], dram["Ct"][:])
            nc.sync.dma_start(Stt[:], dram["St"][:])
            wk = wpool.tile([128, KT, CHG], F16, tag="wres", name="wk")
            half_load(wk, dram["wkT"], slice(0, 4))
            half_load(ct, dram["cc"], slice(0, 4), slice(0, TC))
            half_load(wk, dram["wkT"], slice(4, 8))
            half_load(ct, dram["cc"], slice(4, 8), slice(0, TC))
            bv_row = io.tile([1, CHG], F32, tag="bv", name="bv_row")
            nc.sync.dma_start(bv_row[:], dram["bv"][:])
            nc.gpsimd.partition_broadcast(bv_bc[:], bv_row[:])
            nc.sync.dma_start(ones_col[:], dram["ones128"][:])
            wv = wpool.tile([128, KT, CHG], F16, tag="wres", name="wv")
            half_load(wv, dram["wvT"], slice(0, 4))
            half_load(wv, dram["wvT"], slice(4, 8))
            half_load(ct, dram["cc"], slice(0, 4), slice(TC, T))
            half_load(ct, dram["cc"], slice(4, 8), slice(TC, T))
            half_load(xt, dram["x"], slice(0, 4), slice(TC, T))
            half_load(xt, dram["x"], slice(4, 8), slice(TC, T))
            bcol["bo"] = io.tile([128, OMT, 1], F32, tag="bo", name="bo")
            nc.sync.dma_start(bcol["bo"][:],
                              dram["bo"].rearrange("(mt p) o -> p mt o", p=128))
            wo_t = wpool.tile([128, MT, C], F16, tag="wres", name="wo_res")
            nc.sync.dma_start(wo_t[:],
                              dram["woT"].rearrange("(k p) o -> p k o", p=128))

            # ---------- projection / rope / v units ----------
            pending_ps = {}

            def proj_half(wt, bn, res, src, m, j, half):
                """First/second k-half of a projection m-tile; the second
                half finishes the PSUM group and adds the bias on ScalarE."""
                tsl = slice(j * TC, (j + 1) * TC)
                csl = slice(m * 128, (m + 1) * 128)
                if half == 0:
                    ps = psq.tile([128, TC], F32, tag="ps_q", name="ps")
                    pending_ps[(id(res), m, j)] = ps
                else:
                    ps = pending_ps.pop((id(res), m, j))
                for k in range(half * 4, half * 4 + 4):
                    nc.tensor.matmul(ps[:], wt[:, k, csl], src[:, k, tsl],
                                     start=(k == 0), stop=(k == KT - 1))
                if half == 1:
                    nc.scalar.add(res[:, m, tsl], ps[:], bcol[bn][:, m])

            def proj_unit(wt, bn, res, src, m, j):
                proj_half(wt, bn, res, src, m, j, 0)
                proj_half(wt, bn, res, src, m, j, 1)

            def rope_unit(res, m, j):
                """res = C.*res + S.*(P res); P as one K=128 matmul."""
                tsl = slice(j * TC, (j + 1) * TC)
                ps2 = psq.tile([128, TC], F32, tag="ps_q", name="ps_shuf")
                nc.tensor.matmul(ps2[:], permTt[:], res[:, m, tsl],
                                 start=True, stop=True)
                t1 = spool.tile([128, TC], F16, tag="rope1")
                t2 = spool.tile([128, TC], F16, tag="rope2")
                nc.vector.tensor_mul(t1[:], ps2[:], Stt[:, tsl])
                nc.vector.tensor_mul(t2[:], res[:, m, tsl], Ctt[:, tsl])
                nc.vector.tensor_add(res[:, m, tsl], t1[:], t2[:])

            vts = [None] * ST

            def v_tile(st):
                vt = io.tile([128, NH, HD + 1], F16, tag=f"vt{st}",
                             name=f"vt{st}")
                pv_ = psq.tile([128, CHG], F32, tag="ps_q", name="v_ps")
                ssl = slice(st * 128, (st + 1) * 128)
                for k in range(KT):
                    nc.tensor.matmul(pv_[:], ct[:, k, ssl], wv[:, k],
                                     start=(k == 0), stop=(k == KT - 1))
                nc.vector.tensor_add(
                    vt[:, :, 0:HD],
                    pv_[:].rearrange("p (h d) -> p h d", h=NH),
                    bv_bc[:].rearrange("p (h d) -> p h d", h=NH))
                nc.vector.tensor_copy(vt[:, :, HD],
                                      ones_col[:].to_broadcast([128, NH]))
                vts[st] = vt

            # ---------- attention ----------
            attTs = {}

            def scores(hp, j, st):
                tsl = slice(j * TC, (j + 1) * TC)
                ssl = slice(st * 128, (st + 1) * 128)
                s = pss.tile([128, 2, TC], F32, tag="sAB", name="sAB")
                nc.tensor.matmul(s[:, 0], kr[0:64, hp, ssl], qr[0:64, hp, tsl],
                                 start=True, stop=True)
                nc.tensor.matmul(s[:, 1], kr[64:128, hp, ssl],
                                 qr[64:128, hp, tsl], start=True, stop=True)
                return s

            def attention(hp, j, fillers=()):
                """Per-head-pair, per-512-col-chunk attention with transposed
                PV. fillers: closures of independent PE work, one consumed per
                s-tile between exp and the exp-dependent PV matmuls. The pva
                (head A) PV group lags one s-tile so the previous pass's
                normalize (which frees the pv PSUM slots, pvb first) overlaps
                this pass's first iterations."""
                if j not in attTs:
                    attTs[j] = spool.tile([128, NT2, CHG], F16, tag="attT",
                                          name=f"attT{j}", bufs=2)
                attT = attTs[j]
                pva = pvp.tile([128, NT2, HD + 1], F32, tag="pva",
                               name=f"pva{hp}{j}")
                pvb = pvp.tile([128, NT2, HD + 1], F32, tag="pvb",
                               name=f"pvb{hp}{j}")
                slots = dict(fillers)
                prev = None
                s_t = scores(hp, j, 0)
                for st in range(ST):
                    p = ppool.tile([128, 2, TC], F16, tag="p", name="p")
                    nc.scalar.activation(p[:], s_t[:], AF.Exp, scale=SCALE)
                    if st + 1 < ST:
                        s_t = scores(hp, j, st + 1)
                    if vts[st] is None:
                        v_tile(st)
                    for f in slots.pop(st, ()):
                        f()
                    # one start/stop group per pv bank: start pending-zeroes
                    # the whole 2KB zero region, first touch of each sub-range
                    # overwrites, later touches accumulate.
                    for tcs in range(NT2):
                        nc.tensor.matmul(pvb[:, tcs],
                                         p[:, 1, tcs * 128:(tcs + 1) * 128],
                                         vts[st][:, 2 * hp + 1],
                                         start=(st == 0 and tcs == 0),
                                         stop=(st == ST - 1 and tcs == NT2 - 1))
                    if prev is not None:
                        pst, pp = prev
                        for tcs in range(NT2):
                            nc.tensor.matmul(pva[:, tcs],
                                             pp[:, 0, tcs * 128:(tcs + 1) * 128],
                                             vts[pst][:, 2 * hp],
                                             start=(pst == 0 and tcs == 0),
                                             stop=False)
                    prev = (st, p)
                pst, pp = prev
                for tcs in range(NT2):
                    nc.tensor.matmul(pva[:, tcs],
                                     pp[:, 0, tcs * 128:(tcs + 1) * 128],
                                     vts[pst][:, 2 * hp],
                                     start=False, stop=(tcs == NT2 - 1))
                for stx in sorted(slots):
                    for f in slots[stx]:
                        f()
                slots.clear()

                def norm_one(pv_t, hl, tcs):
                    h = 2 * hp + hl
                    rec = spool.tile([128, 1], F32, tag="rec", name="rec",
                                     bufs=4)
                    nc.vector.reciprocal(rec[:], pv_t[:, tcs, HD:HD + 1])
                    nc.vector.tensor_scalar_mul(
                        attT[:, tcs, h * HD:(h + 1) * HD],
                        pv_t[:, tcs, 0:HD], rec[:])

                if hp == NPAIR - 1:
                    # last head-pair of this chunk: go t-chunk-major and kick
                    # each DMA-xbar transpose the moment its column is done.
                    for tcs in range(NT2):
                        norm_one(pvb, 1, tcs)
                        norm_one(pva, 0, tcs)
                        t0 = j * TC + tcs * 128
                        nc.sync.dma_start_transpose(att[:, :, t0:t0 + 128],
                                                    attT[:, tcs, :])
                else:
                    # pvb first: frees the next pass's first PV slot early
                    for pv_t, hl in ((pvb, 1), (pva, 0)):
                        for tcs in range(NT2):
                            norm_one(pv_t, hl, tcs)

            def o_unit_tc(m, j, final=False):
                """O-projection m-tile emitted per 128-col chunk so each chunk
                only waits on its own transpose. final=True: chunk biases
                alternate ScalarE/DVE and one full-width write drains last."""
                osl = slice(m * 128, (m + 1) * 128)
                tsl = slice(j * TC, (j + 1) * TC)
                ot = opool.tile([128, TC], F32, tag="o_sb", name="ot")
                for tcs in range(NT2):
                    csl = slice(tcs * 128, (tcs + 1) * 128)
                    dsl = slice(j * TC + tcs * 128, j * TC + (tcs + 1) * 128)
                    po = psq.tile([128, TC], F32, tag="ps_q", name="po")
                    for k in range(MT):
                        nc.tensor.matmul(po[:, 0:128], wo_t[:, k, osl],
                                         att[:, k, dsl],
                                         start=(k == 0), stop=(k == MT - 1))
                    if final and tcs % 2:
                        nc.vector.tensor_scalar_add(ot[:, csl], po[:, 0:128],
                                                    bcol["bo"][:, m])
                    else:
                        nc.scalar.add(ot[:, csl], po[:, 0:128],
                                      bcol["bo"][:, m])
                    if not final:
                        nc.sync.dma_start(out[osl, dsl], ot[:, csl])
                if final:
                    nc.sync.dma_start(out[osl, tsl], ot[:])

            def o_unit(m, j, bias_act=False, out_act=False):
                """O-projection m-tile. bias_act: bias-add on ScalarE (for
                units past the exp stream). out_act: write-out from the ACT
                HWDGE queue, keeping SP clear for transposes."""
                tsl = slice(j * TC, (j + 1) * TC)
                osl = slice(m * 128, (m + 1) * 128)
                po = psq.tile([128, TC], F32, tag="ps_q", name="po")
                for k in range(MT):
                    nc.tensor.matmul(po[:], wo_t[:, k, osl], att[:, k, tsl],
                                     start=(k == 0), stop=(k == MT - 1))
                ot = opool.tile([128, TC], F32, tag="o_sb", name="ot")
                if bias_act:
                    nc.scalar.add(ot[:], po[:], bcol["bo"][:, m])
                else:
                    nc.vector.tensor_scalar_add(ot[:], po[:], bcol["bo"][:, m])
                (nc.scalar if out_act else nc.sync).dma_start(out[osl, tsl],
                                                              ot[:])

            # ---------- schedule ----------
            pending_po = {}

            def o_half(m, j, h, bias_act=False):
                tsl = slice(j * TC, (j + 1) * TC)
                osl = slice(m * 128, (m + 1) * 128)
                if h == 0:
                    po = psq.tile([128, TC], F32, tag="ps_q", name="po")
                    pending_po[(m, j)] = po
                else:
                    po = pending_po.pop((m, j))
                for k in (0, 1) if h == 0 else (2, 3):
                    nc.tensor.matmul(po[:], wo_t[:, k, osl], att[:, k, tsl],
                                     start=(k == 0), stop=(k == MT - 1))
                if h == 1:
                    ot = opool.tile([128, TC], F32, tag="o_sb", name="ot")
                    if bias_act:
                        nc.scalar.add(ot[:], po[:], bcol["bo"][:, m])
                    else:
                        nc.vector.tensor_scalar_add(ot[:], po[:],
                                                    bcol["bo"][:, m])
                    nc.sync.dma_start(out[osl, tsl], ot[:])

            def qH(m, j, h):
                return lambda: proj_half(wq, "bq", qr, xt, m, j, h)

            def qR(m, j):
                return lambda: rope_unit(qr, m, j)

            def kH(m, j, h):
                return lambda: proj_half(wk, "bk", kr, ct, m, j, h)

            def kR(m, j):
                return lambda: rope_unit(kr, m, j)

            def oH(m, j, h):
                return lambda: o_half(m, j, h)

            # pre-attention: q j0 m-tiles first (gated only by wq + x j0),
            # then ropes (Ct/St/permT arrive early), then k m0 j0 (c j0).
            for m in range(MT):
                proj_unit(wq, "bq", qr, xt, m, 0)
            for m in range(MT):
                rope_unit(qr, m, 0)
            proj_half(wk, "bk", kr, ct, 0, 0, 0)
            proj_half(wk, "bk", kr, ct, 0, 0, 1)
            rope_unit(kr, 0, 0)

            # k m0 j1-cols + q m0 j1 ride inside (0,0) (v-projection pass);
            # the k(0,1) halves go first so kr m0 s-cols 512: are roped
            # before s-tile 4 consumes them (c j1 lands after wv).
            attention(0, 0, {
                0: [kH(0, 1, 0)], 1: [kH(0, 1, 1)], 2: [kR(0, 1)],
                4: [qH(0, 1, 0)], 5: [qH(0, 1, 1)], 6: [qR(0, 1)],
            })
            attention(0, 1, {
                0: [kH(1, 0, 0)], 1: [kH(1, 0, 1)], 2: [kR(1, 0)],
                4: [kH(1, 1, 0)], 6: [kH(1, 1, 1)],
            })
            attention(1, 0, {
                0: [kR(1, 1)], 1: [qH(1, 1, 0)], 3: [qH(1, 1, 1)],
                4: [qR(1, 1)], 5: [kH(2, 0, 0)], 6: [kH(2, 0, 1)],
            })
            attention(1, 1, {
                0: [kR(2, 0)], 2: [kH(2, 1, 0)], 4: [kH(2, 1, 1)],
            })
            attention(2, 0, {
                0: [kR(2, 1)], 2: [qH(2, 1, 0)], 4: [qH(2, 1, 1)],
                5: [qR(2, 1)],
            })
            attention(2, 1, {
                0: [kH(3, 0, 0)], 1: [kH(3, 0, 1)], 2: [kR(3, 0)],
                4: [kH(3, 1, 0)], 6: [kH(3, 1, 1)],
            })
            attention(3, 0, {
                0: [kR(3, 1)], 2: [qH(3, 1, 0)], 4: [qH(3, 1, 1)],
                5: [qR(3, 1)],
            })
            attention(3, 1, {
                0: [oH(0, 0, 0), oH(0, 0, 1)], 1: [oH(1, 0, 0), oH(1, 0, 1)],
                2: [oH(2, 0, 0)], 3: [oH(2, 0, 1), oH(3, 0, 0)],
                4: [oH(3, 0, 1), oH(4, 0, 0)], 5: [oH(4, 0, 1)],
                6: [oH(5, 0, 0)], 7: [oH(5, 0, 1)],
            })
            o_half(6, 0, 0)
            o_half(6, 0, 1, bias_act=True)
            o_half(7, 0, 0)
            o_half(7, 0, 1, bias_act=True)
            o_unit_tc(0, 1)
            o_unit_tc(1, 1)
            for m in range(2, OMT - 1):
                o_unit(m, 1, bias_act=True)
            o_unit_tc(OMT - 1, 1, final=True)
    nc.finalize()
    return nc


def _get_runner():
    """Build the Bass program once, wrap it in a cached jitted shard_map
    callable (mirrors bass2jax.run_bass_via_pjrt)."""
    if "runner" in _cache:
        return _cache["runner"]

    import jax
    from jax.sharding import Mesh, PartitionSpec, NamedSharding
    from jax.experimental.shard_map import shard_map
    from concourse import bass2jax, mybir

    bass2jax.install_neuronx_cc_hook()
    nc = _build_nc()

    partition_name = (nc.partition_id_tensor.name
                      if nc.partition_id_tensor else None)
    in_names, out_names, out_avals, zero_shapes = [], [], [], []
    for alloc in nc.m.functions[0].allocations:
        if not isinstance(alloc, mybir.MemoryLocationSet):
            continue
        name = alloc.memorylocations[0].name
        if alloc.kind == "ExternalInput":
            if name != partition_name:
                in_names.append(name)
        elif alloc.kind == "ExternalOutput":
            shape = tuple(alloc.tensor_shape)
            dtype = mybir.dt.np(alloc.dtype)
            out_names.append(name)
            out_avals.append(jax.core.ShapedArray(shape, dtype))
            zero_shapes.append((shape, dtype))
    n_params = len(in_names)
    all_names = list(in_names) + list(out_names)
    if partition_name is not None:
        all_names.append(partition_name)
    donate = tuple(range(n_params, n_params + len(out_names)))

    def _body(*args):
        operands = list(args)
        if partition_name is not None:
            operands.append(bass2jax.partition_id_tensor())
        outs = bass2jax._bass_exec_p.bind(
            *operands,
            out_avals=tuple(out_avals),
            in_names=tuple(all_names),
            out_names=tuple(out_names),
            lowering_input_output_aliases=(),
            sim_require_finite=True,
            sim_require_nnan=True,
            nc=nc,
        )
        return tuple(outs)

    devices = jax.devices()[:NCORES]
    mesh = Mesh(np.asarray(devices), ("core",))
    n_out = len(out_names)
    in_specs = (PartitionSpec("core"),) * (n_params + n_out)
    out_specs = (PartitionSpec("core"),) * n_out
    sharded = jax.jit(
        shard_map(_body, mesh=mesh, in_specs=in_specs, out_specs=out_specs,
                  check_rep=False),
        donate_argnums=donate, keep_unused=True)
    core_sharding = NamedSharding(mesh, PartitionSpec("core"))

    import jax.numpy as jnp
    zeros_fn = jax.jit(
        lambda: tuple(jnp.zeros((NCORES * s[0], *s[1:]), d)
                      for s, d in zero_shapes),
        out_shardings=tuple(core_sharding for _ in zero_shapes))

    class Runner:
        _zeros_jit = staticmethod(zeros_fn)

        def device_put(self, in_maps):
            """Place each core's shard directly on its device (no host
            concat of the global array)."""
            placed = []
            for name in in_names:
                shards = [
                    jax.device_put(np.asarray(m[name]), d)
                    for m, d in zip(in_maps, devices)
                ]
                shape0 = shards[0].shape
                placed.append(jax.make_array_from_single_device_arrays(
                    (NCORES * shape0[0], *shape0[1:]), core_sharding, shards))
            return placed

        def zeros(self):
            return self._zeros_jit()

        def execute(self, placed):
            out = sharded(*placed, *self.zeros())
            jax.block_until_ready(out)
            return out

        def __call__(self, in_maps):
            t0 = time.perf_counter()
            placed = self.device_put(in_maps)
            t1 = time.perf_counter()
            out_arrs = self.execute(placed)
            t2 = time.perf_counter()
            self.last_transfer_s = t1 - t0
            self.last_exec_s = t2 - t1
            self.last_wall_s = t2 - t0
            return [
                {name: np.asarray(out_arrs[i]).reshape(NCORES, *out_avals[i].shape)[c]
                 for i, name in enumerate(out_names)}
                for c in range(NCORES)
            ]

    runner = Runner()
    _cache["runner"] = runner
    return runner


def _prep_in_maps(x, c, Wq, bq, Wk, bk, Wv, bv, Wo, bo):
    Ct, St = _cs_tiles()
    x = np.asarray(x, dtype=np.float32).astype(BF)
    c = np.asarray(c, dtype=np.float32).astype(BF)
    shared = {
        "Ct": Ct.astype(BF), "St": St.astype(BF),
        "permT": _perm_matrix().astype(BF),
        "ones128": np.ones((128, 1), dtype=np.float32),
    }
    # weight prep depends only on the head-group, not the batch
    per_group = []
    for g in range(GROUPS):
        gsl = slice(g * CHG, (g + 1) * CHG)
        per_group.append({
            "wqT": np.ascontiguousarray(Wq[gsl].T).astype(BF),
            "wkT": np.ascontiguousarray(Wk[gsl].T).astype(BF),
            "wvT": np.ascontiguousarray(Wv[gsl].T).astype(BF),
            "woT": np.ascontiguousarray(Wo[:, gsl].T).astype(BF),
            "bq": bq[gsl][:, None].astype(np.float32),
            "bk": bk[gsl][:, None].astype(np.float32),
            "bv": bv[gsl][None, :].astype(np.float32),
            "bo": (bo[:, None] if g == 0
                   else np.zeros((C, 1))).astype(np.float32),
            **shared,
        })
    return [
        {"x": np.ascontiguousarray(x[b]), "cc": np.ascontiguousarray(c[b]),
         **per_group[g]}
        for b in range(B) for g in range(GROUPS)
    ]


def kernel(x, c, attn_mask, Wq, bq, Wk, bk, Wv, bv, Wo, bo):
    # attn_mask is all-ones per the problem spec; the where() in the
    # reference is a no-op, so it is not applied on-device.
    runner = _get_runner()
    in_maps = _prep_in_maps(np.asarray(x), np.asarray(c),
                            np.asarray(Wq), np.asarray(bq),
                            np.asarray(Wk), np.asarray(bk),
                            np.asarray(Wv), np.asarray(bv),
                            np.asarray(Wo), np.asarray(bo))
    results = runner(in_maps)
    out = np.empty((B, C, T), dtype=np.float32)
    for b in range(B):
        out[b] = results[2 * b]["out"] + results[2 * b + 1]["out"]
    return out
